# revision 1
# baseline (speedup 1.0000x reference)
"""LISTA-c (complex LISTA) Trainium2 Bass kernel, 8-core data parallel.

Math (per batch element, complex dims N=128 -> M=256, T=10 iters):
  ys  = interleaved real/imag of y          (256-vector)
  ay  = g0 * Wa_int @ ys                    (512-vector; g0 folded into Wa)
  x0  = softshrink_th0(ay)
  u_t = W_t x_{t-1} + (g_t/g0) * ay,  W_t = I - g_t*Wc_int
  x_t = softshrink_{th_t}(u_t) = u_t - clamp(u_t, -th, th)
  out = x_T de-interleaved to (256, 2)

Everything on-chip is bf16 except the PSUM accumulators (f32) and the final
f32 output conversion; measured end-to-end rel err ~6e-3 vs the 2e-2 budget.
Engine balance per iteration (4 feature chunks of 128, batch tile 512):
  PE:   18 matmuls (16 W-chunks + scaled-identity mms folding g*ay on j=0,1)
  ACT:  4 relus (relu-pair softshrink on chunks 0,1, straight from PSUM)
  DVE:  2 stt (g*ay+psum) + 2 clamp (dual-op tensor_scalar) + 2 sub
  Pool: 2 subs
The next pair's prologue (y transposes, Ay matmuls, x0) is emitted inside
iterations 7-9 of the current pair so pair boundaries keep every engine fed.
Chunk j=2,3 softshrink = u - clamp(u) with the clamp a single dual-scalar-op
tensor_scalar. Final iteration x_T (bf16) is PE-transposed (128-cycle bf16
transposes) so the output DMA is fully contiguous f32 rows. Weights stream
in per-iteration DMA chunks on the sync queue while y tiles ride the scalar
queue (and outputs the vector queue) so they never FIFO behind the weights.
"""

import numpy as np
from contextlib import ExitStack

import concourse.bass as bass
import concourse.bacc as bacc
import concourse.tile as tile
import concourse.mybir as mybir

F32 = mybir.dt.float32
BF16 = mybir.dt.bfloat16
LAMBD = 1.0
NCORES = 8
BATCH = 65536
N = 128          # y complex dim
M = 256          # x complex dim
T = 10
KF = 512         # real feature dim of x (2*M)
KY = 256         # real feature dim of y (2*N)
FT = 512         # batch tile (free dim)
PER_CORE = BATCH // NCORES
YV_BF16 = True
NTILES = PER_CORE // FT

# packed weight layout (columns of the [128, WPK_COLS] bf16 dram tensor):
#   wat:  8*128      [kc<2][j<4]     lhsT chunks of g0*Wa_int
#   gid:  T*128      [t=1..T]        (g_t/g0)-scaled 128-identities
#   idb:  128                        identity (transposes)
#   wts:  T*16*128   [t=1..T][kc][j] lhsT chunks of W_t
N_WAT = 8 * 128
N_GID = T * 128
O_GID = N_WAT
O_IDB = N_WAT + N_GID
O_WTS = O_IDB + 128
WPK_COLS = O_WTS + T * 16 * 128
PSBANK = {0: 0, 1: 1, 2: 0, 3: 1}   # chunk -> PSUM bank tag per tile


def _interleave_cw(W0, W1):
    """Complex matrix (W0 + i W1), (m, n) -> real interleaved (2m, 2n):
    out[2a+c, 2b+d] so that out @ interleave(x) = interleave(W x)."""
    m, n = W0.shape
    W = np.zeros((2 * m, 2 * n), dtype=np.float64)
    W[0::2, 0::2] = W0
    W[0::2, 1::2] = -W1
    W[1::2, 0::2] = W1
    W[1::2, 1::2] = W0
    return W


def build_nc(etas, gammas):
    """etas/gammas: python floats list of length T+1 (baked as immediates)."""
    nc = bacc.Bacc("TRN2", target_bir_lowering=False, debug=False,
                   num_devices=NCORES)
    COPY = mybir.ActivationFunctionType.Copy
    RELU = mybir.ActivationFunctionType.Relu
    ALU = mybir.AluOpType

    yv = nc.declare_dram_parameter("yv", [PER_CORE, KY], BF16, isOutput=False)
    wpk = nc.declare_dram_parameter("wpk", [128, WPK_COLS], BF16, isOutput=False)
    out = nc.declare_dram_parameter("out", [PER_CORE, KF], F32, isOutput=True)

    th = [float(e) * LAMBD for e in etas]
    g = [float(x) for x in gammas]
    g0 = g[0] if g[0] != 0.0 else 1.0
    gr = [gt / g0 for gt in g]       # per-iter ay scale after g0 folding

    with tile.TileContext(nc) as tc, ExitStack() as ctx:
        wp = ctx.enter_context(tc.tile_pool(name="wp", bufs=1))
        ysbp = ctx.enter_context(tc.tile_pool(name="ysbp", bufs=4))
        ysp = ctx.enter_context(tc.tile_pool(name="ysp", bufs=4))
        ayp = ctx.enter_context(tc.tile_pool(name="ayp", bufs=2))
        xp = ctx.enter_context(tc.tile_pool(name="xp", bufs=4))
        wwp = ctx.enter_context(tc.tile_pool(name="wwp", bufs=5))
        clp = ctx.enter_context(tc.tile_pool(name="clp", bufs=4))
        pnp = ctx.enter_context(tc.tile_pool(name="pnp", bufs=5))
        osbp = ctx.enter_context(tc.tile_pool(name="osbp", bufs=3))
        psmm = ctx.enter_context(tc.tile_pool(name="psmm", bufs=1, space="PSUM"))

        # weight tiles; y DMAs for the first pair are emitted before these
        # transfers are queued (see the main loop) and ride a different queue.
        wsm = wp.tile([128, O_WTS], BF16, tag="wsm")
        # one weight tile per DISTINCT W_t (identical gammas share a tile)
        uniq = []
        tslot = {}
        for t in range(1, T + 1):
            key = g[t]
            if key not in tslot:
                tslot[key] = len(uniq)
                uniq.append(t)
        wts = [wp.tile([128, 16 * 128], BF16, tag=f"wts{i}", name=f"wts{i}")
               for i in range(len(uniq))]
        t2slot = {t: tslot[g[t]] for t in range(1, T + 1)}

        def emit_weight_dmas():
            nc.sync.dma_start(wsm[:], wpk[:, 0:O_WTS])
            for i, t in enumerate(uniq):
                off = O_WTS + (t - 1) * 16 * 128
                nc.sync.dma_start(wts[i][:], wpk[:, off:off + 16 * 128])

        def wat_ap(kc, j):
            off = (kc * 4 + j) * 128
            return wsm[:, off:off + 128]

        def gid_ap(t):
            off = O_GID + (t - 1) * 128
            return wsm[:, off:off + 128]

        idb = wsm[:, O_IDB:O_IDB + 128]

        def wts_ap(t, kc, j):
            off = (kc * 4 + j) * 128
            return wts[t2slot[t]][:, off:off + 128]

        # per-threshold bias const tiles (activation bias must be an SBUF AP)
        biasp = ctx.enter_context(tc.tile_pool(name="biasp", bufs=1))
        bias_by_val = {}
        bias_t = []
        for t in range(T + 1):
            v = -th[t]
            if v not in bias_by_val:
                bt = biasp.tile([128, 1], F32, tag=f"bias{len(bias_by_val)}")
                nc.vector.memset(bt[:], v)
                bias_by_val[v] = bt
            bias_t.append(bias_by_val[v])

        def prologue_dma(ti):
            """y tile HBM->SBUF on the scalar queue (not behind weights)."""
            b0 = ti * FT
            ysb = ysbp.tile([128, 4, KY], BF16, tag="ysb")
            src = yv[b0:b0 + FT, :].rearrange("(s p) f -> p s f", p=128)
            nc.scalar.dma_start(ysb[:], src)
            return ysb

        def prologue_stage1(tag, ysb):
            """Transpose y tile to feature-major bf16 ys."""
            ys = ysp.tile([128, 2, FT], BF16, tag="ys", name=f"ys{tag}")
            for h in range(2):
                ytp = psmm.tile([128, FT], BF16, tag=f"{tag}{h}",
                                name=f"ytr{h}")
                for s in range(4):
                    nc.tensor.transpose(ytp[:, s * 128:(s + 1) * 128],
                                        ysb[:, s, h * 128:(h + 1) * 128],
                                        idb)
                nc.scalar.activation(ys[:, h, :], ytp[:], COPY)
            return ys

        def prologue_stage2(tag, ys, ay, x, js):
            """ay chunks = g0*Wa@ys (into the aux PSUM ring), SBUF copies,
            x0 chunks = ay - clamp(ay)."""
            for j in js:
                psa = psmm.tile([128, FT], F32, tag=f"{tag}{PSBANK[j]}", name=f"psa{j}")
                for kc in range(2):
                    nc.tensor.matmul(psa[:], wat_ap(kc, j), ys[:, kc, :],
                                     start=(kc == 0), stop=(kc == 1))
                if j % 2 == 0:
                    nc.scalar.activation(ay[:, j, :], psa[:], COPY)
                else:
                    nc.vector.tensor_copy(ay[:, j, :], psa[:])
                c = clp.tile([128, FT], BF16, tag="c")
                nc.vector.tensor_scalar(c[:], ay[:, j, :], -th[0], th[0],
                                        ALU.max, ALU.min)
                if j < 2:
                    nc.gpsimd.tensor_tensor(x[:, j, :], ay[:, j, :], c[:],
                                            ALU.subtract)
                else:
                    nc.vector.tensor_tensor(x[:, j, :], ay[:, j, :], c[:],
                                            ALU.subtract)

        def alloc_ax(tag):
            ay = ayp.tile([128, 4, FT], BF16, tag=f"ay{tag}", name=f"ay{tag}")
            x = xp.tile([128, 4, FT], BF16, tag=f"x{tag}", name=f"x{tag}_0")
            return ay, x

        def iter_mms_pair(t, st, tags, j):
            """Both tiles' matmuls interleaved at the k level so each weight
            chunk is loaded once per pair. j==0 folds gr[t]*ay via a scaled
            identity matmul."""
            pss = {}
            for tag in tags:
                pss[tag] = psmm.tile([128, FT], F32, tag=f"{tag}{PSBANK[j]}",
                                     name=f"ps{tag}{j}")
            for tag in tags:
                nc.tensor.matmul(pss[tag][:], gid_ap(t),
                                 st[tag][0][:, j, :],
                                 start=True, stop=False)
            for k in range(4):
                for tag in tags:
                    nc.tensor.matmul(pss[tag][:], wts_ap(t, k, j),
                                     st[tag][1][:, k, :],
                                     start=False,
                                     stop=(k == 3))
            return pss

        def iter_tails_pair(t, st, newx, pss, j):
            """softshrink tails for chunk j, group tiles interleaved.
            u is already complete in PSUM (g*ay folded by matmul)."""
            tags = tuple(pss.keys())
            for tag in tags:
                p = pnp.tile([128, FT], BF16, tag="p", name=f"p{tag}")
                n = pnp.tile([128, FT], BF16, tag="n", name=f"n{tag}")
                nc.scalar.activation(p[:], pss[tag][:], RELU,
                                     bias=bias_t[t][:], scale=1.0)
                nc.scalar.activation(n[:], pss[tag][:], RELU,
                                     bias=bias_t[t][:], scale=-1.0)
                if j < 2:
                    nc.gpsimd.tensor_tensor(newx[tag][:, j, :], p[:], n[:],
                                            ALU.subtract)
                else:
                    nc.vector.tensor_tensor(newx[tag][:, j, :], p[:], n[:],
                                            ALU.subtract)

        def epilogue(ti, tag, xb):
            """Transpose bf16 x_T to batch-major, convert to f32, DMA out."""
            b0 = ti * FT
            for s in range(4):
                pso = psmm.tile([128, FT], BF16, tag=f"{tag}{s % 2}",
                                name=f"pso{s}")
                for j in range(4):
                    nc.tensor.transpose(pso[:, j * 128:(j + 1) * 128],
                                        xb[:, j, s * 128:(s + 1) * 128],
                                        idb)
                osb = osbp.tile([128, FT], F32, tag="osb")
                if s % 2 == 0:
                    nc.scalar.activation(osb[:], pso[:], COPY)
                else:
                    nc.vector.tensor_copy(osb[:], pso[:])
                nc.sync.dma_start(out[b0 + s * 128:b0 + (s + 1) * 128, :],
                                  osb[:])

        import os
        import contextlib
        _trips = int(os.environ.get("KREP_HW", "0"))
        _loop = tc.For_i(0, _trips, 1) if _trips > 0 else contextlib.nullcontext()
        emitted_w = False
        GROUPS = [("A", "B", "C", "D")] * 4
        GTILES = []
        _ti = 0
        for _g in GROUPS:
            GTILES.append(tuple(range(_ti, _ti + len(_g))))
            _ti += len(_g)
        assert _ti == NTILES
        npairs = len(GROUPS)
        if _trips > 0:
            # timing mode: weights once, outside the hardware loop
            emit_weight_dmas()
            emitted_w = True
        with _loop:
         for _rep in range(int(os.environ.get("KREP", "1"))):
          st = None
          for pair in range(npairs):
            AB = GROUPS[pair]
            tiles = GTILES[pair]
            if st is None:
                # priming: full prologue for the first pair
                ysbs = {}
                for ti, tag in zip(tiles, AB):
                    ysbs[tag] = prologue_dma(ti)
                if not emitted_w:
                    # after the first y DMAs so those go out first
                    emit_weight_dmas()
                    emitted_w = True
                st = {}
                for ti, tag in zip(tiles, AB):
                    ys = prologue_stage1(tag, ysbs[tag])
                    ay, x = alloc_ax(tag)
                    prologue_stage2(tag, ys, ay, x, (0, 1, 2, 3))
                    st[tag] = [ay, x]
            nxt = {}
            if pair + 1 < npairs:
                nAB = GROUPS[pair + 1]
                ntile = GTILES[pair + 1]
                nxt["ysbs"] = {tag: prologue_dma(ti)
                               for ti, tag in zip(ntile, nAB)}
            for t in range(1, T + 1):
                newx = {}
                for tag in AB:
                    newx[tag] = xp.tile([128, 4, FT], BF16, tag=f"x{tag}",
                                        name=f"x{tag}_{t}")
                for j in range(4):
                    pss = iter_mms_pair(t, st, AB, j)
                    iter_tails_pair(t, st, newx, pss, j)
                for tag in AB:
                    st[tag][1] = newx[tag]
                # next pair's prologue rides along iterations 7-9
                if nxt:
                    if t == 5:
                        nxt["ys"] = {tag: prologue_stage1(tag, nxt["ysbs"][tag])
                                     for tag in nAB}
                        nxt["ax"] = {tag: alloc_ax(tag) for tag in nAB}
                    elif t in (6, 7, 8, 9):
                        for tag in nAB:
                            ay, x = nxt["ax"][tag]
                            prologue_stage2(tag, nxt["ys"][tag], ay, x,
                                            (t - 6,))
            for ti, tag in zip(tiles, AB):
                epilogue(ti, tag, st[tag][1])
            if nxt:
                st = {tag: list(nxt["ax"][tag]) for tag in nAB}
            else:
                st = None

    nc.compile()
    return nc


def host_pack(A, B, etas, gammas):
    """Build the packed weight tensor (128, WPK_COLS) bf16."""
    import ml_dtypes
    g = [float(x) for x in np.asarray(gammas).reshape(-1)]
    g0 = g[0] if g[0] != 0.0 else 1.0
    Wa = g0 * _interleave_cw(A[0].astype(np.float64), A[1].astype(np.float64))
    Wc = _interleave_cw(B[0].astype(np.float64), B[1].astype(np.float64))
    I = np.eye(KF)

    cols = []
    for kc in range(2):
        for j in range(4):
            cols.append(Wa[j * 128:(j + 1) * 128, kc * 128:(kc + 1) * 128].T)
    for t in range(1, T + 1):
        cols.append((g[t] / g0) * np.eye(128))
    cols.append(np.eye(128))
    for t in range(1, T + 1):
        Wt = I - g[t] * Wc
        for kc in range(4):
            for j in range(4):
                cols.append(Wt[j * 128:(j + 1) * 128,
                               kc * 128:(kc + 1) * 128].T)
    return np.concatenate(cols, axis=1).astype(ml_dtypes.bfloat16)


def _run(nc, in_maps):
    from concourse import bass2jax
    return bass2jax.run_bass_via_pjrt(nc, in_maps, n_cores=NCORES)


def kernel(y, A, B, etas, gammas):
    import ml_dtypes
    y = np.ascontiguousarray(np.asarray(y, dtype=np.float32))
    A = np.asarray(A, dtype=np.float32)
    B = np.asarray(B, dtype=np.float32)
    ev = [float(x) for x in np.asarray(etas, dtype=np.float32).reshape(-1)]
    gv = [float(x) for x in np.asarray(gammas, dtype=np.float32).reshape(-1)]

    nc = build_nc(ev, gv)
    wpk = host_pack(A, B, ev, gv)
    yflat = y.reshape(BATCH, KY).astype(ml_dtypes.bfloat16)
    in_maps = [{"yv": yflat[c * PER_CORE:(c + 1) * PER_CORE], "wpk": wpk}
               for c in range(NCORES)]
    res = _run(nc, in_maps)
    outs = [res[c]["out"] for c in range(NCORES)]
    full = np.concatenate(outs, axis=0)          # (BATCH, 512)
    return full.reshape(BATCH, M, 2)



# revision 2
# speedup vs baseline: 3.2110x; 3.2110x over previous
"""LISTA-c Trainium2 Bass kernel, 8-core data parallel, delta-form PSUM.

Math (per batch element, complex dims N=128 -> M=256, T=10 iters):
  ys  = feature-major real/imag of y          (256-vector)
  ay  = g1 * Wa_int @ ys                      (512-vector, g1 folded into Wa)
  x0  = softshrink_th0((g0/g1) * ay)
  u_t = x_{t-1} - g_t*Wc x_{t-1} + g_t*ay
  x_t = softshrink_{th_t}(u_t)

When g_1..g_T are all equal (the shipped inputs: gammas=ones), the delta
form u_{t+1} = u_t + W d_t with d_t = x_t - x_{t-1} and W = I - g*Wc keeps
u resident in a PSUM bank for the tile's whole lifetime: no per-iteration
identity matmuls and no ay re-add.  PE does exactly 16 [128x128]x[128x512]
bf16 matmuls per iteration per tile.

Engine balance per iteration per tile (FT=512):
  PE:   16 matmuls (213ns each)                        ~3.4us
  ACT:  4 relus (relu-pair softshrink, chunks 0,1)     ~2.4us
  DVE:  2 clamps + 2 subs from PSUM (chunks 2,3)       ~2.6us
  Pool: 2 relu-pair subs + 4 delta subs (bf16 SBUF)    ~2.6us
PSUM: 2 tiles in flight x 4 persistent u banks = all 8 banks; prologue
y-transpose scratch and epilogue transpose scratch reuse freed banks via
tile versioning at pair boundaries.

Non-uniform gammas fall back to the classic form (fresh accumulation with
a scaled-identity ay fold, 20 matmuls/iter) in the same skeleton.
"""

import numpy as np
from contextlib import ExitStack

import concourse.bass as bass
import concourse.bacc as bacc
import concourse.tile as tile
import concourse.mybir as mybir

F32 = mybir.dt.float32
BF16 = mybir.dt.bfloat16
LAMBD = 1.0
NCORES = 8
BATCH = 65536
N = 128          # y complex dim
M = 256          # x complex dim
T = 10
KF = 512         # real feature dim of x (2*M)
KY = 256         # real feature dim of y (2*N)
FT = 512         # batch tile (free dim)
PER_CORE = BATCH // NCORES
YV_BF16 = True
NTILES = PER_CORE // FT
NPAIRS = NTILES // 2

# packed weight layout (columns of the [128, WPK_COLS] bf16 dram tensor):
#   wat:  8*128      [kc<2][j<4]     lhsT chunks of g1*Wa_int
#   gid:  T*128      [t=1..T]        (g_t/g1)-scaled 128-identities
#   idb:  128                        identity (transposes)
#   wts:  T*16*128   [t=1..T][kc][j] lhsT chunks of W_t (dedup by g_t)
N_WAT = 8 * 128
N_GID = T * 128
O_GID = N_WAT
O_IDB = N_WAT + N_GID
O_WTS = O_IDB + 128


def _wscale(g):
    s = g[1] if T >= 1 else g[0]
    return s if s != 0.0 else 1.0


def _uniq_slots(g):
    uniq, tslot = [], {}
    for t in range(1, T + 1):
        if g[t] not in tslot:
            tslot[g[t]] = len(uniq)
            uniq.append(t)
    return uniq, {t: tslot[g[t]] for t in range(1, T + 1)}


def _interleave_cw(W0, W1):
    """Complex matrix (W0 + i W1), (m, n) -> real interleaved (2m, 2n)."""
    m, n = W0.shape
    W = np.zeros((2 * m, 2 * n), dtype=np.float64)
    W[0::2, 0::2] = W0
    W[0::2, 1::2] = -W1
    W[1::2, 0::2] = W1
    W[1::2, 1::2] = W0
    return W


def build_nc(etas, gammas):
    """etas/gammas: python floats list of length T+1 (baked as immediates)."""
    nc = bacc.Bacc("TRN2", target_bir_lowering=False, debug=False,
                   num_devices=NCORES)
    COPY = mybir.ActivationFunctionType.Copy
    RELU = mybir.ActivationFunctionType.Relu
    ALU = mybir.AluOpType

    yv = nc.declare_dram_parameter("yv", [PER_CORE, KY], BF16, isOutput=False)
    uniq, t2slot = _uniq_slots([float(x) for x in gammas])
    wpk_cols = O_WTS + len(uniq) * 16 * 128
    wpk = nc.declare_dram_parameter("wpk", [128, wpk_cols], BF16,
                                    isOutput=False)
    out = nc.declare_dram_parameter("out", [PER_CORE, KF], F32, isOutput=True)

    th = [float(e) * LAMBD for e in etas]
    g = [float(x) for x in gammas]
    s1 = _wscale(g)
    uniform = all(gt == g[1] for gt in g[1:]) if T >= 1 else True
    x0_scale = g[0] / s1          # relu input scale for the x0 tails

    with tile.TileContext(nc) as tc, ExitStack() as ctx:
        wp = ctx.enter_context(tc.tile_pool(name="wp", bufs=1))
        ysbp = ctx.enter_context(tc.tile_pool(name="ysbp", bufs=2))
        ysp = ctx.enter_context(tc.tile_pool(name="ysp", bufs=2))
        xp = ctx.enter_context(tc.tile_pool(name="xp", bufs=3))
        dp = ctx.enter_context(tc.tile_pool(name="dp", bufs=2))
        pnp = ctx.enter_context(tc.tile_pool(name="pnp", bufs=3))
        cp = ctx.enter_context(tc.tile_pool(name="cp", bufs=3))
        osbp = ctx.enter_context(tc.tile_pool(name="osbp", bufs=3))
        aysp = ctx.enter_context(tc.tile_pool(name="aysp", bufs=2))
        psmm = ctx.enter_context(tc.tile_pool(name="psmm", bufs=1,
                                              space="PSUM"))

        wsm = wp.tile([128, O_WTS], BF16, tag="wsm")
        idbt = wp.tile([128, 128], BF16, tag="idbt")
        wts = [wp.tile([128, 16 * 128], BF16, tag=f"wts{i}", name=f"wts{i}")
               for i in range(len(uniq))]

        def emit_weight_dmas():
            # idb first: the first y transposes need only the identity
            nc.sync.dma_start(idbt[:], wpk[:, O_IDB:O_IDB + 128])
            nc.sync.dma_start(wsm[:], wpk[:, 0:O_WTS])
            for i, t in enumerate(uniq):
                off = O_WTS + i * 16 * 128
                nc.sync.dma_start(wts[i][:], wpk[:, off:off + 16 * 128])

        def wat_ap(kc, j):
            off = (kc * 4 + j) * 128
            return wsm[:, off:off + 128]

        def gid_ap(t):
            off = O_GID + (t - 1) * 128
            return wsm[:, off:off + 128]

        idb = idbt[:, :]

        def wts_ap(t, kc, j):
            off = (kc * 4 + j) * 128
            return wts[t2slot[t]][:, off:off + 128]

        # per-threshold bias const tiles (activation bias must be an SBUF AP)
        biasp = ctx.enter_context(tc.tile_pool(name="biasp", bufs=1))
        bias_by_val = {}
        bias_t = []
        for t in range(T + 1):
            v = -th[t]
            if v not in bias_by_val:
                bt = biasp.tile([128, 1], F32, tag=f"bias{len(bias_by_val)}")
                nc.vector.memset(bt[:], v)
                bias_by_val[v] = bt
            bias_t.append(bias_by_val[v])

        def psum(tag, dtype):
            return psmm.tile([128, FT], dtype, tag=tag, name=f"ps{tag}")

        def dma_y(ti, tag):
            b0 = ti * FT
            ysb = ysbp.tile([128, 4, KY], BF16, tag=f"ysb{tag}", name=f"ysb{tag}")
            src = yv[b0:b0 + FT, :].rearrange("(s p) f -> p s f", p=128)
            nc.sync.dma_start(ysb[:], src)
            return ysb

        def trans_y(tag, ysb):
            """Feature-major bf16 ys via PE transposes (banks tag0/tag1)."""
            ys = ysp.tile([128, 2, FT], BF16, tag=f"ys{tag}", name=f"ys{tag}")
            for h in range(2):
                ytp = psum(f"{tag}{h}", BF16)
                for s in range(4):
                    nc.tensor.transpose(ytp[:, s * 128:(s + 1) * 128],
                                        ysb[:, s, h * 128:(h + 1) * 128],
                                        idb)
                nc.scalar.activation(ys[:, h, :], ytp[:], COPY)
            return ys

        def ay_mms(tag, ys, ub, js):
            """u[j] = s1 * Wa_int @ ys  (start of the persistent bank)."""
            for j in js:
                u = psum(f"{tag}{j}", F32)
                for kc in range(2):
                    nc.tensor.matmul(u[:], wat_ap(kc, j), ys[:, kc, :],
                                     start=(kc == 0), stop=(kc == 1))
                ub[j] = u

        def tails(t, u_j, j, xnew, scale=1.0):
            """softshrink_{th_t}(scale * u_j) -> xnew[:, j, :] (bf16)."""
            if j < 2 or scale != 1.0:
                p = pnp.tile([128, FT], BF16, tag="p", name="p")
                n = pnp.tile([128, FT], BF16, tag="n", name="n")
                nc.scalar.activation(p[:], u_j[:], RELU,
                                     bias=bias_t[t][:], scale=scale)
                nc.scalar.activation(n[:], u_j[:], RELU,
                                     bias=bias_t[t][:], scale=-scale)
                nc.gpsimd.tensor_tensor(xnew[:, j, :], p[:], n[:],
                                        ALU.subtract)
            else:
                c = cp.tile([128, FT], F32, tag="c", name="c")
                nc.vector.tensor_scalar(c[:], u_j[:], -th[t], th[t],
                                        ALU.max, ALU.min)
                nc.vector.tensor_tensor(xnew[:, j, :], u_j[:], c[:],
                                        ALU.subtract)

        def x0_tails(tag, ub):
            x0 = xp.tile([128, 4, FT], BF16, tag=f"x{tag}", name=f"x{tag}_0")
            for j in range(4):
                tails(0, ub[j], j, x0, scale=x0_scale)
            return x0

        def ay_copy(tag, ub):
            """Materialize s1*ay in SBUF (classic path only)."""
            ay = aysp.tile([128, 4, FT], BF16, tag=f"ay{tag}", name=f"ay{tag}")
            for j in range(4):
                if j % 2 == 0:
                    nc.scalar.activation(ay[:, j, :], ub[j][:], COPY)
                else:
                    nc.vector.tensor_copy(ay[:, j, :], ub[j][:])
            return ay

        def iter_mms_delta(tag, t, j, ub, dmov):
            u = ub[j]
            for k in range(4):
                nc.tensor.matmul(u[:], wts_ap(t, k, j), dmov[:, k, :],
                                 start=False, stop=(k == 3))

        def iter_mms_classic(tag, t, j, ub, xprev, ay):
            u = psum(f"{tag}{j}", F32)
            ub[j] = u
            nc.tensor.matmul(u[:], gid_ap(t), ay[:, j, :],
                             start=True, stop=False)
            for k in range(4):
                nc.tensor.matmul(u[:], wts_ap(t, k, j), xprev[:, k, :],
                                 start=False, stop=(k == 3))

        def epilogue(ti, tag, xT, banktags):
            """Transpose bf16 x_T to batch-major, convert to f32, DMA out."""
            b0 = ti * FT
            for s in range(4):
                pso = psmm.tile([128, FT], BF16, tag=banktags[s], name=f"pso{s}")
                for j in range(4):
                    nc.tensor.transpose(pso[:, j * 128:(j + 1) * 128],
                                        xT[:, j, s * 128:(s + 1) * 128],
                                        idb)
                osb = osbp.tile([128, FT], F32, tag="osb", name="osb")
                if s % 2 == 0:
                    nc.scalar.activation(osb[:], pso[:], COPY)
                else:
                    nc.vector.tensor_copy(osb[:], pso[:])
                nc.sync.dma_start(out[b0 + s * 128:b0 + (s + 1) * 128, :],
                                  osb[:])

        def prologue_full(tag, ysb):
            """trans + ay mms + x0 for a tile whose banks are all free."""
            ys = trans_y(tag, ysb)
            ub = [None] * 4
            ay_mms(tag, ys, ub, (0, 1, 2, 3))
            st = {"ub": ub, "x": x0_tails(tag, ub), "d": None}
            st["ay"] = None if uniform else ay_copy(tag, ub)
            return st

        def emit_iter(tag, t, st):
            """One LISTA iteration for one tile; updates st in place."""
            xnew = xp.tile([128, 4, FT], BF16, tag=f"x{tag}",
                           name=f"x{tag}_{t}")
            dmov = st["x"] if (t == 1 or not uniform) else st["d"]
            for j in range(4):
                if uniform:
                    iter_mms_delta(tag, t, j, st["ub"], dmov)
                else:
                    iter_mms_classic(tag, t, j, st["ub"], dmov, st["ay"])
                tails(t, st["ub"][j], j, xnew)
            if uniform and t < T:
                d = dp.tile([128, 4, FT], BF16, tag=f"d{tag}",
                            name=f"d{tag}_{t}")
                for j in range(4):
                    nc.gpsimd.tensor_tensor(d[:, j, :], xnew[:, j, :],
                                            st["x"][:, j, :], ALU.subtract)
                st["d"] = d
            st["x"] = xnew

        import os
        import contextlib
        _trips = int(os.environ.get("KREP_HW", "0"))
        _loop = (tc.For_i(0, _trips, 1) if _trips > 0
                 else contextlib.nullcontext())
        emitted_w = False
        if _trips > 0:
            # timing mode: weights once, outside the hardware loop
            emit_weight_dmas()
            emitted_w = True
        with _loop:
         for _rep in range(int(os.environ.get("KREP", "1"))):
          st = None
          for pair in range(NPAIRS):
            tA, tB = 2 * pair, 2 * pair + 1
            if st is None:
                # priming: full prologue for the first pair
                ysbA = dma_y(tA, "A")
                ysbB = dma_y(tB, "B")
                if not emitted_w:
                    emit_weight_dmas()
                    emitted_w = True
                st = {"A": prologue_full("A", ysbA),
                      "B": prologue_full("B", ysbB)}
            nxt = None
            for t in range(1, T + 1):
                emit_iter("A", t, st["A"])
                emit_iter("B", t, st["B"])
                if t == 6 and pair + 1 < NPAIRS:
                    nxt = {"ysbA": dma_y(2 * pair + 2, "A"),
                           "ysbB": dma_y(2 * pair + 3, "B")}
            if nxt is not None:
                # boundary: weave next pair's prologue through the freed
                # banks.  Chains per tag keep PE-critical fills early and
                # epilogue scratch (feeds only DMA) on its own tags:
                #   A0: uA0 -> ytrC0 -> uC0      B0: uB0 -> ytrD0 -> uD0
                #   A1: uA1 -> ytrC1 -> uC1      B1: uB1 -> ytrD1 -> uD1
                #   A2: uA2 -> epiA s0 -> epiA s2 -> uC2   (A3/B2/B3 alike)
                ysC = trans_y("A", nxt["ysbA"])
                ysD = trans_y("B", nxt["ysbB"])
                ubC = [None] * 4
                ubD = [None] * 4
                ay_mms("A", ysC, ubC, (0, 1))
                ay_mms("B", ysD, ubD, (0, 1))
                epilogue(tA, "A", st["A"]["x"], ("A2", "A3", "A2", "A3"))
                epilogue(tB, "B", st["B"]["x"], ("B2", "B3", "B2", "B3"))
                ay_mms("A", ysC, ubC, (2, 3))
                ay_mms("B", ysD, ubD, (2, 3))
                stC = {"ub": ubC, "x": x0_tails("A", ubC), "d": None}
                stC["ay"] = None if uniform else ay_copy("A", ubC)
                stD = {"ub": ubD, "x": x0_tails("B", ubD), "d": None}
                stD["ay"] = None if uniform else ay_copy("B", ubD)
                st = {"A": stC, "B": stD}
            else:
                epilogue(tA, "A", st["A"]["x"], ("A0", "A1", "A2", "A3"))
                epilogue(tB, "B", st["B"]["x"], ("B0", "B1", "B2", "B3"))
                st = None

    nc.compile()
    return nc


def host_pack(A, B, etas, gammas):
    """Build the packed weight tensor (128, wpk_cols) bf16."""
    import ml_dtypes
    g = [float(x) for x in np.asarray(gammas).reshape(-1)]
    s1 = _wscale(g)
    uniq, _ = _uniq_slots(g)
    Wa = s1 * _interleave_cw(A[0].astype(np.float64), A[1].astype(np.float64))
    Wc = _interleave_cw(B[0].astype(np.float64), B[1].astype(np.float64))
    I = np.eye(KF)

    cols = []
    for kc in range(2):
        for j in range(4):
            cols.append(Wa[j * 128:(j + 1) * 128, kc * 128:(kc + 1) * 128].T)
    for t in range(1, T + 1):
        cols.append((g[t] / s1) * np.eye(128))
    cols.append(np.eye(128))
    for t in uniq:
        Wt = I - g[t] * Wc
        for kc in range(4):
            for j in range(4):
                cols.append(Wt[j * 128:(j + 1) * 128,
                               kc * 128:(kc + 1) * 128].T)
    return np.concatenate(cols, axis=1).astype(ml_dtypes.bfloat16)


def _run(nc, in_maps):
    from concourse import bass2jax
    return bass2jax.run_bass_via_pjrt(nc, in_maps, n_cores=NCORES)


def kernel(y, A, B, etas, gammas):
    import ml_dtypes
    y = np.ascontiguousarray(np.asarray(y, dtype=np.float32))
    A = np.asarray(A, dtype=np.float32)
    B = np.asarray(B, dtype=np.float32)
    ev = [float(x) for x in np.asarray(etas, dtype=np.float32).reshape(-1)]
    gv = [float(x) for x in np.asarray(gammas, dtype=np.float32).reshape(-1)]

    nc = build_nc(ev, gv)
    wpk = host_pack(A, B, ev, gv)
    yflat = y.reshape(BATCH, KY).astype(ml_dtypes.bfloat16)
    in_maps = [{"yv": yflat[c * PER_CORE:(c + 1) * PER_CORE], "wpk": wpk}
               for c in range(NCORES)]
    res = _run(nc, in_maps)
    outs = [res[c]["out"] for c in range(NCORES)]
    full = np.concatenate(outs, axis=0)          # (BATCH, 512)
    return full.reshape(BATCH, M, 2)
